# revision 9
# baseline (speedup 1.0000x reference)
"""Sparse cross-attention (squared-ReLU normalizer) on 8 TRN2 NeuronCores.

Sharding: 8 cores = batch(2) x head-group(4). Each core owns one batch and
4 of 16 heads (a 256-wide slice of hsize): Wq/Wkv column-parallel,
Wo row-parallel (partial outputs summed on host), mask replicated per
batch shard.

v2 design (per-core, all matmuls bf16 w/ fp32 PSUM):
  Stage A: rqT (hs,q), rkT (hs,s) projections; rv (s, 65-per-head w/ ones
    col) for AV-stationary use. Evictions batched over 2 PSUM banks.
  Main loop per q-tile (512) x head-pair:
    scores: 2x ROW-TILED matmuls (K=adim=64; heads 2p/2p+1 in PE rows
      0-63/64-127 concurrently) -> S^T [s128, q512] fp32 PSUM
    elementwise, routed across ACT/DVE/GpSimd (sc-pair granularity):
      route B (sc 0-11):  ACT relu-evict r = relu(S+nb)      [P,1024] ops
                          u2 = r * maskT   [TT mult, GpSimd sc0-7 / DVE]
      route A (sc 12-15): DVE u = (S+nb)*maskT  [scalar_tensor_tensor]
                          v = max(u,0)          [tensor_scalar 4x]
      square: t = v*v over the whole head tile  [TT mult 2x, DVE]
    AV: rv slice [128, 65] STATIONARY (col 64 = ones -> denominator row),
      tT moving N=512, accumulate 16 s-chunks -> po [65, q512] = [oT_h; den]
    normalize: den -> bf16 (+eps), PE K=1 broadcast to 64 rows, DVE
      reciprocal, fused scale into oT_sb (bf16) at the head's partitions.
    Wo: oT chunk stationary [128,128], woT moving, accumulate HC=2 -> out.
"""

import numpy as np
import ml_dtypes

BF16 = ml_dtypes.bfloat16

B, Q, S, D = 2, 2048, 2048, 1024
NUM_HEAD, ADIM = 16, 64
HSIZE = NUM_HEAD * ADIM
N_CORES = 8
GROUPS = 4                  # head groups (tensor-parallel dim)
HPG = NUM_HEAD // GROUPS    # 4 heads per core
HS = HPG * ADIM             # 256: per-core hsize slice
IEPS = 1e-32
P = 128

# elementwise routing (sc-pair granularity, 8 pairs of s-chunks):
# pairs < ROUTE_B_PAIRS go through ACT relu-evict; rest through DVE fused.
ROUTE_B_PAIRS = 6          # 12 s-chunks on ACT, 4 on DVE
GP_MUL_SC = 8              # first N route-B s-chunks' mask-mul on GpSimd

_COMPILED = None


def _build(q=Q, s=S, d=D, hpg=HPG, adim=ADIM, qt=512):
    """Build + compile the per-core Bass program. Returns the Bacc."""
    from contextlib import ExitStack
    import concourse.bass as bass
    import concourse.mybir as mybir
    import concourse.tile as tile
    from concourse import bacc

    fp32 = mybir.dt.float32
    bf16 = mybir.dt.bfloat16
    ALU = mybir.AluOpType
    AF = mybir.ActivationFunctionType

    hs = hpg * adim
    DC = d // P          # contraction chunks for projections
    NQ = q // qt         # q tiles
    SC = s // P          # s chunks
    HC = hs // P         # hsize-slice chunks (2)
    assert hs % P == 0 and q % qt == 0 and d % 512 == 0

    nc = bacc.Bacc("TRN2", target_bir_lowering=False, debug=False,
                   num_devices=N_CORES)

    qT = nc.dram_tensor("qT", [d, q], bf16, kind="ExternalInput").ap()
    kT = nc.dram_tensor("kT", [d, s], bf16, kind="ExternalInput").ap()
    wqT = nc.dram_tensor("wqT", [d, hs], bf16, kind="ExternalInput").ap()
    wkT = nc.dram_tensor("wkT", [d, hs], bf16, kind="ExternalInput").ap()
    wvT = nc.dram_tensor("wvT", [d, hs], bf16, kind="ExternalInput").ap()
    woT = nc.dram_tensor("woT", [hs, d], bf16, kind="ExternalInput").ap()
    maskT = nc.dram_tensor("maskT", [s, q], bf16, kind="ExternalInput").ap()
    nbias = nc.dram_tensor("nbias", [1, 1], fp32, kind="ExternalInput").ap()
    out = nc.dram_tensor("out", [q, d], fp32, kind="ExternalOutput").ap()

    qT_t = qT.rearrange("(c p) q -> c p q", p=P)      # [DC, 128, q]
    kT_t = kT.rearrange("(c p) s -> c p s", p=P)
    wqT_t = wqT.rearrange("(c p) h -> c p h", p=P)
    wkT_t = wkT.rearrange("(c p) h -> c p h", p=P)
    wvT_t = wvT.rearrange("(c p) h -> c p h", p=P)
    woT_t = woT.rearrange("(c p) d -> c p d", p=P)    # [HC, 128, d]
    maskT_t = maskT.rearrange("(c p) q -> p c q", p=P)  # [128, SC, q]
    out_t = out.rearrange("(c p) d -> c p d", p=P)    # [q/P, 128, d]

    with tile.TileContext(nc) as tc, ExitStack() as ctx:
        const = ctx.enter_context(tc.tile_pool(name="const", bufs=1))
        wpool = ctx.enter_context(tc.tile_pool(name="w", bufs=1))
        xpool = ctx.enter_context(tc.tile_pool(name="x", bufs=10))
        actp = ctx.enter_context(tc.tile_pool(name="act", bufs=1))
        maskp = ctx.enter_context(tc.tile_pool(name="mask", bufs=2))
        up = ctx.enter_context(tc.tile_pool(name="u", bufs=2))
        tTp = ctx.enter_context(tc.tile_pool(name="tT", bufs=2))
        oTp = ctx.enter_context(tc.tile_pool(name="oT", bufs=2))
        denp = ctx.enter_context(tc.tile_pool(name="den", bufs=4))
        recp = ctx.enter_context(tc.tile_pool(name="rec", bufs=4))
        outp = ctx.enter_context(tc.tile_pool(name="out", bufs=3))
        psS = ctx.enter_context(tc.tile_pool(name="psS", bufs=2, space="PSUM"))
        psO = ctx.enter_context(tc.tile_pool(name="psO", bufs=2, space="PSUM"))
        psA = ctx.enter_context(tc.tile_pool(name="psA", bufs=2, space="PSUM"))

        # ---- constants ----
        ones1 = const.tile([1, P], fp32)
        nc.any.memset(ones1[:], 1.0)
        ones64 = const.tile([1, 64], bf16)
        nc.any.memset(ones64[:], 1.0)
        nb1 = const.tile([1, 1], fp32)
        nc.sync.dma_start(nb1[:], nbias[:])
        # broadcast nbias to all 128 partitions via K=1 matmul outer product
        ps_nb = psA.tile([P, 512], fp32, tag="ps_a")
        nc.tensor.matmul(ps_nb[:, 0:1], ones1[:], nb1[:], start=True, stop=True)
        nb128 = const.tile([P, 1], fp32)
        nc.scalar.copy(nb128[:], ps_nb[:, 0:1])

        # ---- resident weights + stage A inputs (interleaved DMA) ----
        wq_sb = wpool.tile([P, DC, hs], bf16)
        wk_sb = wpool.tile([P, DC, hs], bf16)
        wv_sb = wpool.tile([P, DC, hs], bf16)
        wo_sb = wpool.tile([P, HC, d], bf16)

        x_tiles = []
        for c in range(DC):
            nc.sync.dma_start(wq_sb[:, c], wqT_t[c])
            xt = xpool.tile([P, q], bf16, tag="xch")
            nc.sync.dma_start(xt[:], qT_t[c])
            x_tiles.append(xt)

        # ---- activations (resident) ----
        rqT_sb = actp.tile([P, HC, q], bf16)    # (hs, q)
        rkT_sb = actp.tile([P, HC, s], bf16)    # (hs, s)
        rv_sb = actp.tile([P, SC, hpg * (adim + 1)], bf16)  # (s, hs + ones)
        nc.any.memset(rv_sb[:], 1.0)            # ones cols survive at 64::65

        scale = 1.0 / np.sqrt(np.float32(adim))

        # ---- stage A1: rqT = (Wq_slice @ iQ[b]^T) scaled ----
        for m in range(HC):
            for nq2 in range(q // 1024):
                ps = psS.tile([P, 1024], fp32, tag="ps_s")
                for half in range(2):
                    nq = 2 * nq2 + half
                    for c in range(DC):
                        nc.tensor.matmul(
                            ps[:, half * 512:(half + 1) * 512],
                            wq_sb[:, c, m * P:(m + 1) * P],
                            x_tiles[c][:, nq * 512:(nq + 1) * 512],
                            start=(c == 0), stop=(c == DC - 1))
                # fold the 1/sqrt(adim) score scale into rq
                nc.scalar.activation(
                    rqT_sb[:, m, nq2 * 1024:(nq2 + 1) * 1024], ps[:],
                    AF.Copy, scale=float(scale))

        # ---- stage A2: rkT and rv (x tiles re-used for kT) ----
        x_tiles = []
        for c in range(DC):
            nc.sync.dma_start(wk_sb[:, c], wkT_t[c])
            nc.sync.dma_start(wv_sb[:, c], wvT_t[c])
            xt = xpool.tile([P, s], bf16, tag="xch")
            nc.sync.dma_start(xt[:], kT_t[c])
            x_tiles.append(xt)
        for c in range(HC):
            nc.sync.dma_start(wo_sb[:, c], woT_t[c])

        for m in range(HC):
            for nq2 in range(s // 1024):
                ps = psS.tile([P, 1024], fp32, tag="ps_s")
                for half in range(2):
                    nq = 2 * nq2 + half
                    for c in range(DC):
                        nc.tensor.matmul(
                            ps[:, half * 512:(half + 1) * 512],
                            wk_sb[:, c, m * P:(m + 1) * P],
                            x_tiles[c][:, nq * 512:(nq + 1) * 512],
                            start=(c == 0), stop=(c == DC - 1))
                nc.scalar.copy(rkT_sb[:, m, nq2 * 1024:(nq2 + 1) * 1024],
                               ps[:])
        for sc4 in range(SC // 4):
            ps = psS.tile([P, 1024], fp32, tag="ps_s")
            for si in range(4):
                sc = 4 * sc4 + si
                for c in range(DC):
                    nc.tensor.matmul(
                        ps[:, si * 256:si * 256 + hs],
                        x_tiles[c][:, sc * P:(sc + 1) * P],
                        wv_sb[:, c], start=(c == 0), stop=(c == DC - 1))
            # scatter heads into 65-strided groups (col 64 of each stays 1.0)
            nc.scalar.copy(
                rv_sb[:, 4 * sc4:4 * sc4 + 4].rearrange(
                    "p s (h c) -> p s h c", c=adim + 1)[:, :, :, 0:adim],
                ps[:].rearrange("p (s h c) -> p s h c", s=4, c=adim))

        # ---- main loop per q tile ----
        mblk0 = maskp.tile([P, SC, qt], bf16, tag="mblk", name="mblk0")
        nc.sync.dma_start(mblk0[:], maskT_t[:, :, 0:qt])
        mblks = {0: mblk0}

        NBP = ROUTE_B_PAIRS

        for iq in range(NQ):
            qlo = iq * qt
            mblk = mblks.pop(iq)
            if iq + 1 < NQ:
                mnext = maskp.tile([P, SC, qt], bf16, tag="mblk",
                                   name=f"mblk{iq+1}")
                nc.sync.dma_start(
                    mnext[:], maskT_t[:, :, (iq + 1) * qt:(iq + 2) * qt])
                mblks[iq + 1] = mnext

            oT_sb = oTp.tile([P, HC, qt], bf16, tag="oT", name=f"oT{iq}")
            tTs = {}

            def scores_pair(p_, iq=iq, qlo=qlo, mblk=mblk, tTs=tTs):
                """Row-tiled scores for heads 2p,2p+1 + fused elementwise."""
                hc = p_
                us = {}
                for hh in (0, 1):
                    h = 2 * p_ + hh
                    tTs[h] = tTp.tile([P, SC, qt], bf16, tag="tT",
                                      name=f"tT{iq}_{h}")
                    us[hh] = up.tile([P, SC, qt], bf16, tag="u",
                                     name=f"u{iq}_{h}")
                for scp in range(SC // 2):   # sc pairs
                    pss = []
                    for hh in (0, 1):
                        ps = psS.tile([P, 1024], fp32, tag="ps_s",
                                      name=f"s{iq}_{p_}_{scp}_{hh}")
                        pss.append(ps)
                    for half in range(2):
                        sc = 2 * scp + half
                        for hh in (0, 1):
                            lo = hh * adim
                            nc.tensor.matmul(
                                pss[hh][:, half * 512:(half + 1) * 512],
                                rkT_sb[lo:lo + adim, hc, sc * P:(sc + 1) * P],
                                rqT_sb[lo:lo + adim, hc, qlo:qlo + qt],
                                start=True, stop=True)
                    for hh in (0, 1):
                        h = 2 * p_ + hh
                        ps, u = pss[hh], us[hh]
                        sc = 2 * scp
                        if scp < NBP:
                            # route B: ACT relu-evict both banks at once
                            nc.scalar.activation(
                                u[:, sc:sc + 2], ps[:], AF.Relu,
                                bias=nb128[:])
                        else:
                            # route A: DVE fused (S+nb)*mask
                            nc.vector.scalar_tensor_tensor(
                                u[:, sc:sc + 2], ps[:], nb128[:],
                                mblk[:, sc:sc + 2], ALU.add, ALU.mult)
                # P2 per head
                for hh in (0, 1):
                    h = 2 * p_ + hh
                    u, tT = us[hh], tTs[h]
                    # route B: u2 = r*mask -> tT
                    if GP_MUL_SC > 0:
                        nc.gpsimd.tensor_tensor(
                            tT[:, 0:GP_MUL_SC], u[:, 0:GP_MUL_SC],
                            mblk[:, 0:GP_MUL_SC], ALU.mult)
                    if GP_MUL_SC < 2 * NBP:
                        nc.vector.tensor_tensor(
                            tT[:, GP_MUL_SC:2 * NBP], u[:, GP_MUL_SC:2 * NBP],
                            mblk[:, GP_MUL_SC:2 * NBP], ALU.mult)
                    # route A: v = max(u,0) -> tT (tensor_scalar, 4x)
                    if NBP < SC // 2:
                        nc.vector.tensor_scalar_max(
                            tT[:, 2 * NBP:SC], u[:, 2 * NBP:SC], 0.0)
                    # square the whole head tile in place
                    nc.vector.tensor_tensor(tT[:], tT[:], tT[:], ALU.mult)

            def av_head(h, iq=iq, qlo=qlo, oT_sb=oT_sb, tTs=tTs):
                tT = tTs.pop(h)
                po = psO.tile([P, qt], fp32, tag="po", name=f"po{iq}_{h}")
                for sc in range(SC):
                    nc.tensor.matmul(
                        po[0:adim + 1, :],
                        rv_sb[:, sc, h * (adim + 1):(h + 1) * (adim + 1)],
                        tT[:, sc], start=(sc == 0), stop=(sc == SC - 1))
                # denominator -> bf16 (+eps), PE broadcast, reciprocal, scale
                den = denp.tile([1, qt], bf16, tag="den", name=f"dn{iq}_{h}")
                nc.scalar.activation(den[:], po[adim:adim + 1, :], AF.Copy,
                                     bias=float(IEPS))
                psb = psA.tile([P, 512], fp32, tag="ps_a", name=f"bc{iq}_{h}")
                nc.tensor.matmul(psb[0:adim, :], ones64[:], den[:],
                                 start=True, stop=True)
                rec = recp.tile([adim, qt], bf16, tag="rec", name=f"rc{iq}_{h}")
                with nc.allow_low_precision(
                        reason="bf16 attn normalizer; rel-err budget 2e-2"):
                    nc.vector.reciprocal(rec[:], psb[0:adim, :])
                hp = (h % 2) * adim
                nc.vector.scalar_tensor_tensor(
                    oT_sb[hp:hp + adim, h // 2, :], po[0:adim, :], 1.0,
                    rec[:], ALU.mult, ALU.mult)

            # software-pipelined emission: scores one pair ahead of AV
            scores_pair(0)
            scores_pair(1)
            av_head(0)
            av_head(1)
            av_head(2)
            av_head(3)

            # Wo: out partial (q, d) = oT^T @ woT
            for qc in range(qt // P):
                for nd in range(d // 512):
                    ps = psA.tile([P, 512], fp32, tag="ps_a",
                                  name=f"wo{iq}_{qc}_{nd}")
                    for c in range(HC):
                        nc.tensor.matmul(
                            ps[:], oT_sb[:, c, qc * P:(qc + 1) * P],
                            wo_sb[:, c, nd * 512:(nd + 1) * 512],
                            start=(c == 0), stop=(c == HC - 1))
                    ob = outp.tile([P, 512], fp32, tag="ob", name="ob_t")
                    if (qc + nd) % 2 == 0:
                        nc.scalar.copy(ob[:], ps[:])
                    else:
                        nc.vector.tensor_copy(ob[:], ps[:])
                    nc.sync.dma_start(
                        out_t[iq * (qt // P) + qc, :, nd * 512:(nd + 1) * 512],
                        ob[:])

    nc.compile()
    return nc


def _shard_inputs(iQ, iK, mask, Wq, Wkv, Wo, nbias):
    in_maps = []
    maskT_by_b = [np.ascontiguousarray((~mask[b]).T).astype(BF16)
                  for b in range(B)]
    qT_by_b = [np.ascontiguousarray(iQ[b].T).astype(BF16) for b in range(B)]
    kT_by_b = [np.ascontiguousarray(iK[b].T).astype(BF16) for b in range(B)]
    nb = np.asarray(nbias, np.float32).reshape(1, 1)
    for ci in range(N_CORES):
        b, g = ci // GROUPS, ci % GROUPS
        hsl = slice(g * HS, (g + 1) * HS)
        in_maps.append({
            "qT": qT_by_b[b],
            "kT": kT_by_b[b],
            "wqT": np.ascontiguousarray(Wq[hsl].T).astype(BF16),
            "wkT": np.ascontiguousarray(Wkv[hsl].T).astype(BF16),
            "wvT": np.ascontiguousarray(Wkv[HSIZE + g * HS:HSIZE + (g + 1) * HS].T).astype(BF16),
            "woT": np.ascontiguousarray(Wo[:, hsl].T).astype(BF16),
            "maskT": maskT_by_b[b],
            "nbias": nb,
        })
    return in_maps


def kernel(iQ, iK, mask, Wq, Wkv, Wo, nbias):
    global _COMPILED
    from concourse.bass_utils import run_bass_kernel_spmd

    if _COMPILED is None:
        _COMPILED = _build()
    in_maps = _shard_inputs(np.asarray(iQ, np.float32), np.asarray(iK, np.float32),
                            np.asarray(mask), np.asarray(Wq, np.float32),
                            np.asarray(Wkv, np.float32), np.asarray(Wo, np.float32),
                            np.asarray(nbias, np.float32))
    res = run_bass_kernel_spmd(_COMPILED, in_maps, list(range(N_CORES))).results
    out = np.zeros((B, Q, D), np.float32)
    for ci in range(N_CORES):
        out[ci // GROUPS] += np.asarray(res[ci]["out"], np.float32)
    return out
